# revision 1
# baseline (speedup 1.0000x reference)
"""Trainium2 Bass kernel for GQA attention with RoPE (dense transformer block).

Strategy (8-way tensor parallel over heads, per the sharding hint):
  - Each of the 8 NeuronCores gets 4 Q heads + 1 KV head (KV head c, Q heads 4c..4c+3).
  - Host pre-transposes x -> xT [DIM, B*S] so activations live feature-major on
    the device; all matmuls then need no on-device transposes of x.
  - Host folds 1/sqrt(HD) into wq and permutes wq/wk head-dim columns to
    [even|odd] order so RoPE becomes two 64-partition halves.
  - Per core: QKV projection (feature-major), RoPE on Q/K, scores computed
    directly transposed (scoresT[t,s] = K^T.T @ Q^T), softmax without max
    subtraction (exp + replicated ones-matmul row sums), PV accumulation into
    O^T (SBUF-resident), then the wo matmul per batch produces token-major
    partial outputs.
  - Host sums the 8 partial outputs (the "all-reduce after wo").

All matmuls run as float32r (fp32 storage streamed at full PE rate).
"""
import math

import numpy as np

import concourse.bass as bass
import concourse.tile as tile
from concourse import bacc, mybir
from concourse.bass_utils import run_bass_kernel_spmd
from concourse.masks import make_identity

B, S, DIM = 2, 2048, 4096
NH, NKV, HD = 32, 8, 128
BS = B * S
NCORES = 8
QH = NH // NCORES          # 4 Q heads per core
DQ = QH * HD               # 512
TCH = 512                  # token chunk
NCH = BS // TCH            # 8 chunks
NKT = DIM // 128           # 32 contraction tiles
P = 128

F32 = mybir.dt.float32
F32R = mybir.dt.float32r
AF = mybir.ActivationFunctionType

_prog_cache = {}
LAST_RESULTS = None


def _build(variant):
    """variant: 'causal' | 'none' | 'general'"""
    nc = bacc.Bacc(None, target_bir_lowering=False)
    xT = nc.dram_tensor("xT", [DIM, BS], F32, kind="ExternalInput")
    wq = nc.dram_tensor("wq", [DIM, DQ], F32, kind="ExternalInput")
    wk = nc.dram_tensor("wk", [DIM, HD], F32, kind="ExternalInput")
    wv = nc.dram_tensor("wv", [DIM, HD], F32, kind="ExternalInput")
    wo = nc.dram_tensor("wo", [DQ, DIM], F32, kind="ExternalInput")
    cosT = nc.dram_tensor("cosT", [64, S], F32, kind="ExternalInput")
    sinT = nc.dram_tensor("sinT", [64, S], F32, kind="ExternalInput")
    emask = None
    if variant != "none":
        emask = nc.dram_tensor("emaskT", [S, S], F32, kind="ExternalInput")
    part = nc.dram_tensor("part", [BS, DIM], F32, kind="ExternalOutput")

    with tile.TileContext(nc) as tc:
        with (
            tc.tile_pool(name="dram", bufs=1, space="DRAM") as dram,
            tc.tile_pool(name="const", bufs=1) as constp,
        ):
            qTd = dram.tile([DQ, BS], F32)

            ident = constp.tile([P, P], F32)
            make_identity(nc, ident)
            ones_f = constp.tile([P, P], F32)
            nc.vector.memset(ones_f[:], 1.0)
            ones = constp.tile([P, P], F32R)
            nc.vector.tensor_copy(ones[:], ones_f[:])

            with tc.tile_pool(name="kv", bufs=1) as kvp:
                KT_sb = kvp.tile([P, BS], F32R)   # rope'd K^T, feature-major
                Vtok = kvp.tile([P, BS], F32R)    # V, token-major 128x128 tiles packed

                # ---------------- Phase A: QKV projection + RoPE ----------------
                qtp_cm = tc.tile_pool(name="qtp", bufs=1)
                qtp = qtp_cm.__enter__()
                with (
                    tc.tile_pool(name="wqp", bufs=1) as wqp,
                    tc.tile_pool(name="csp", bufs=1) as csp,
                    tc.tile_pool(name="xtp", bufs=1) as xtp,
                    tc.tile_pool(name="rp", bufs=1) as rp,
                    tc.tile_pool(name="psA", bufs=1, space="PSUM") as psA,
                ):
                    BF16 = mybir.dt.bfloat16
                    cos_sb = csp.tile([64, S], BF16)
                    nc.gpsimd.dma_start(cos_sb[:], cosT[:, :])
                    sin_sb = csp.tile([64, S], BF16)
                    nc.gpsimd.dma_start(sin_sb[:], sinT[:, :])

                    wq3 = wq[:, :].rearrange("(ko p) n -> p ko n", p=P)
                    wk3 = wk[:, :].rearrange("(ko p) n -> p ko n", p=P)
                    wv3 = wv[:, :].rearrange("(ko p) n -> p ko n", p=P)
                    srcs = [wq3[:, :, m * P:(m + 1) * P] for m in range(QH)]
                    srcs += [wk3, wv3]
                    w_sb = [wqp.tile([P, NKT, P], F32R, tag=f"w{m}", bufs=1,
                                     name=f"w_sb{m}")
                            for m in range(6)]
                    KC = 4

                    qt_tiles = {}

                    def _load_qt_pair(b_, hg_):
                        for h_ in (2 * hg_, 2 * hg_ + 1):
                            qt_ = qtp.tile([P, S], F32R, tag="qt", bufs=2,
                                           name=f"qt{h_}_{b_}")
                            for jq_ in (3, 2, 1, 0):
                                nc.sync.dma_start(
                                    qt_[:, jq_ * TCH:(jq_ + 1) * TCH],
                                    qTd[h_ * P:(h_ + 1) * P,
                                        b_ * S + jq_ * TCH:
                                        b_ * S + (jq_ + 1) * TCH].bitcast(F32R))
                            qt_tiles[(b_, h_)] = qt_

                    for tcn in range(NCH):
                        if tcn == 4:
                            _load_qt_pair(0, 0)
                        if tcn == 6:
                            _load_qt_pair(0, 1)
                        cb = tcn % (NCH // B)      # chunk index within batch
                        acc = [psA.tile([P, TCH], F32, tag="acc", bufs=7,
                                        name=f"acc{m}_{tcn}")
                               for m in range(6)]
                        for k in range(NKT):
                            if tcn == 0 and k % KC == 0:
                                # stream weight chunks two groups ahead of use
                                if k == 0:
                                    for m in range(6):
                                        nc.sync.dma_start(
                                            w_sb[m][:, 0:KC, :],
                                            srcs[m][:, 0:KC, :].bitcast(F32R))
                                elif k + 2 * KC <= NKT:
                                    kc = k + KC
                                    for m in range(6):
                                        nc.sync.dma_start(
                                            w_sb[m][:, kc:kc + KC, :],
                                            srcs[m][:, kc:kc + KC, :].bitcast(F32R))
                            if tcn == 0 and k == 1:
                                for m in range(6):
                                    nc.sync.dma_start(
                                        w_sb[m][:, KC:2 * KC, :],
                                        srcs[m][:, KC:2 * KC, :].bitcast(F32R))
                            xt = xtp.tile([P, TCH], F32R, tag="xt", bufs=4,
                                          name=f"xt_{tcn}_{k}")
                            nc.sync.dma_start(
                                xt[:], xT[k * P:(k + 1) * P,
                                          tcn * TCH:(tcn + 1) * TCH].bitcast(F32R))
                            for m in range(6):
                                nc.tensor.matmul(acc[m][:], w_sb[m][:, k, :], xt[:],
                                                 start=(k == 0), stop=(k == NKT - 1))
                        cs = cos_sb[:, cb * TCH:(cb + 1) * TCH]
                        sn = sin_sb[:, cb * TCH:(cb + 1) * TCH]

                        def _emit_v(tcn):
                            vch = rp.tile([P, TCH], F32, tag="vch", bufs=2,
                                          name=f"vch_{tcn}")
                            nc.scalar.copy(vch[:], acc[5][:])
                            for j in range(TCH // P):
                                tp_ps = psA.tile([P, P], F32, tag="tp", bufs=1,
                                                 name=f"tp_{tcn}_{j}")
                                nc.tensor.transpose(
                                    tp_ps[:], vch[:, j * P:(j + 1) * P], ident[:])
                                g = tcn * 4 + j
                                nc.scalar.copy(Vtok[:, g * P:(g + 1) * P], tp_ps[:])

                        if tcn == NCH - 1:
                            _emit_v(tcn)
                        stages = []
                        for m in range(5):
                            slo = rp.tile([64, TCH], F32, tag="slo", bufs=5,
                                          name=f"slo{m}_{tcn}")
                            shi = rp.tile([64, TCH], F32, tag="shi", bufs=5,
                                          name=f"shi{m}_{tcn}")
                            if m % 2 == 0 or tcn == NCH - 1:
                                nc.scalar.copy(slo[:], acc[m][0:64, :])
                                nc.scalar.copy(shi[:], acc[m][64:P, :])
                            else:
                                nc.vector.tensor_copy(slo[:], acc[m][0:64, :])
                                nc.vector.tensor_copy(shi[:], acc[m][64:P, :])
                            stages.append((slo, shi))
                        def _emit_rope(tcn, stages, cs, sn):
                            for m in range(5):
                                slo, shi = stages[m]
                                if m < QH:
                                    out_t = rp.tile([P, TCH], F32R, tag="qo",
                                                    bufs=3, name=f"qo{m}_{tcn}")
                                    o_lo, o_hi = out_t[0:64, :], out_t[64:P, :]
                                else:
                                    ksl = KT_sb[:, tcn * TCH:(tcn + 1) * TCH]
                                    o_lo, o_hi = ksl[0:64, :], ksl[64:P, :]
                                tA = rp.tile([64, TCH], F32, tag="tA", bufs=2,
                                             name=f"tA{m}_{tcn}")
                                tB = rp.tile([64, TCH], F32, tag="tB", bufs=2,
                                             name=f"tB{m}_{tcn}")
                                nc.vector.tensor_mul(tA[:], slo[:], cs)
                                nc.vector.tensor_mul(tB[:], shi[:], sn)
                                nc.vector.tensor_sub(o_lo, tA[:], tB[:])
                                tC = rp.tile([64, TCH], F32, tag="tC", bufs=2,
                                             name=f"tC{m}_{tcn}")
                                tD = rp.tile([64, TCH], F32, tag="tD", bufs=2,
                                             name=f"tD{m}_{tcn}")
                                nc.vector.tensor_mul(tC[:], slo[:], sn)
                                nc.vector.tensor_mul(tD[:], shi[:], cs)
                                nc.vector.tensor_add(o_hi, tC[:], tD[:])
                                if m < QH:
                                    nc.sync.dma_start(
                                        qTd[m * P:(m + 1) * P,
                                            tcn * TCH:(tcn + 1) * TCH].bitcast(F32R),
                                        out_t[:])

                        _emit_rope(tcn, stages, cs, sn)
                        if tcn != NCH - 1:
                            _emit_v(tcn)

                # ------------- Phase B+C interleaved per batch -------------
                with (
                    tc.tile_pool(name="ebp", bufs=1) as ebp,
                    tc.tile_pool(name="mkp", bufs=1) as mkp,
                    tc.tile_pool(name="obp", bufs=1) as obp,
                    tc.tile_pool(name="wop", bufs=1) as wop,
                    tc.tile_pool(name="osb", bufs=1) as osbp,
                    tc.tile_pool(name="psB", bufs=1, space="PSUM") as psB,
                    tc.tile_pool(name="psC", bufs=1, space="PSUM") as psC,
                ):
                    wo_sb = []
                    for kk in range(DQ // P):
                        t2 = wop.tile([P, DIM], F32R, tag=f"wo{kk}", bufs=1,
                                      name=f"wo_sb{kk}")
                        nc.sync.dma_start(
                            t2[:], wo[kk * P:(kk + 1) * P, :].bitcast(F32R))
                        wo_sb.append(t2)

                    for b in range(B):
                        # O^T for this batch, SBUF-resident: one tile per head
                        O_sb = [osbp.tile([P, S], F32R, tag=f"osb{h}", bufs=1,
                                          name=f"osb{h}_{b}")
                                for h in range(QH)]
                        # ---- Phase B: attention for batch b ----
                        for hg in range(QH // 2):
                            hs = [2 * hg, 2 * hg + 1]
                            qts = [qt_tiles[(b, h)] for h in hs]
                            for sc in (3, 2, 1, 0):
                                if sc == 2 and (b, hg) != (0, 0):
                                    nxt = (b, hg + 1) if hg == 0 else (b + 1, 0)
                                    if nxt[0] < B and nxt not in (
                                            (0, 1),) and (nxt[0], 2 * nxt[1]) not in [
                                            k for k in qt_tiles]:
                                        _load_qt_pair(*nxt)
                                o_ps = [psB.tile([P, TCH], F32, tag=f"o{i}", bufs=1,
                                                 name=f"o{i}_{b}_{hg}_{sc}")
                                        for i in range(2)]
                                s_ps = [psB.tile([P, TCH], F32, tag=f"s{i}", bufs=1,
                                                 name=f"s{i}_{b}_{hg}_{sc}")
                                        for i in range(2)]
                                ntt = 4 * sc + 4 if variant == "causal" else 16
                                if variant == "causal":
                                    tt_order = list(range(4 * sc)) + \
                                        list(range(4 * sc, ntt))
                                else:
                                    tt_order = list(range(ntt))
                                first_tt, last_tt = tt_order[0], tt_order[-1]
                                for tt in tt_order:
                                    masked = (variant == "general") or (
                                        variant == "causal" and tt >= 4 * sc)
                                    if masked:
                                        mt = mkp.tile([P, TCH], F32, tag="mg", bufs=3,
                                                      name=f"m_{b}_{hg}_{sc}_{tt}")
                                        nc.sync.dma_start(
                                            mt[:], emask[tt * P:(tt + 1) * P,
                                                         sc * TCH:(sc + 1) * TCH])
                                    g = b * 16 + tt
                                    for i in range(2):
                                        sc_ps = psB.tile([P, TCH], F32, tag="sc", bufs=2)
                                        nc.tensor.matmul(
                                            sc_ps[:],
                                            KT_sb[:, b * S + tt * P: b * S + (tt + 1) * P],
                                            qts[i][:, sc * TCH:(sc + 1) * TCH],
                                            start=True, stop=True)
                                        et = ebp.tile([P, TCH], F32R, tag="et", bufs=6)
                                        if masked:
                                            etm = ebp.tile([P, TCH], F32, tag="etm",
                                                           bufs=2)
                                            nc.scalar.activation(etm[:], sc_ps[:], AF.Exp)
                                            meng = (nc.gpsimd
                                                    if (b, hg, sc) == (0, 0, 3)
                                                    else nc.vector)
                                            meng.tensor_mul(et[:], etm[:], mt[:])
                                        else:
                                            nc.scalar.activation(et[:], sc_ps[:], AF.Exp)
                                        nc.tensor.matmul(
                                            o_ps[i][:], Vtok[:, g * P:(g + 1) * P], et[:],
                                            start=(tt == first_tt),
                                            stop=(tt == last_tt))
                                        nc.tensor.matmul(
                                            s_ps[i][:], ones[:], et[:],
                                            start=(tt == first_tt),
                                            stop=(tt == last_tt))
                                for i, h in enumerate(hs):
                                    rec = obp.tile([P, TCH], F32, tag="rec", bufs=2)
                                    nc.vector.reciprocal(rec[:], s_ps[i][:])
                                    nc.vector.tensor_mul(
                                        O_sb[h][:, sc * TCH:(sc + 1) * TCH],
                                        o_ps[i][:], rec[:])

                        # ---- Phase C: wo projection for batch b ----
                        for tt in reversed(range(S // P)):
                            for nn in range(DIM // TCH):
                                pp = psC.tile([P, TCH], F32, tag="pc", bufs=2)
                                for kk in range(DQ // P):
                                    nc.tensor.matmul(
                                        pp[:], O_sb[kk][:, tt * P:(tt + 1) * P],
                                        wo_sb[kk][:, nn * TCH:(nn + 1) * TCH],
                                        start=(kk == 0), stop=(kk == DQ // P - 1))
                                ob = obp.tile([P, TCH], F32, tag="obc", bufs=4)
                                if nn % 2 == 0:
                                    nc.vector.tensor_copy(ob[:], pp[:])
                                else:
                                    nc.scalar.copy(ob[:], pp[:])
                                nc.sync.dma_start(
                                    part[b * S + tt * P: b * S + (tt + 1) * P,
                                         nn * TCH:(nn + 1) * TCH], ob[:])
                qtp_cm.__exit__(None, None, None)

    nc.compile()
    return nc


def _get_prog(variant):
    if variant not in _prog_cache:
        _prog_cache[variant] = _build(variant)
    return _prog_cache[variant]


def prepare(inputs):
    """Host-side sharding prep: returns (variant, program, per-core input maps)."""
    x = np.asarray(inputs["x"], dtype=np.float32)
    wq = np.asarray(inputs["wq"], dtype=np.float32)
    wk = np.asarray(inputs["wk"], dtype=np.float32)
    wv = np.asarray(inputs["wv"], dtype=np.float32)
    wo = np.asarray(inputs["wo"], dtype=np.float32)
    fc = np.asarray(inputs["freqs_cos"], dtype=np.float32)
    fs = np.asarray(inputs["freqs_sin"], dtype=np.float32)
    mask = np.asarray(inputs["mask"], dtype=np.float32)

    xT = np.ascontiguousarray(x.reshape(BS, DIM).T)
    perm = np.concatenate([np.arange(0, HD, 2), np.arange(1, HD, 2)])
    wq_p = (wq.reshape(DIM, NH, HD)[:, :, perm] / math.sqrt(HD)).astype(np.float32)
    wk_p = wk.reshape(DIM, NKV, HD)[:, :, perm].astype(np.float32)
    cosT = np.ascontiguousarray(fc.T)
    sinT = np.ascontiguousarray(fs.T)

    if not mask.any():
        variant = "none"
    else:
        il, jl = np.tril_indices(S)
        iu, ju = np.triu_indices(S, 1)
        if np.all(mask[il, jl] == 0.0) and np.all(mask[iu, ju] <= -1e8):
            variant = "causal"
        else:
            variant = "general"
    emaskT = None
    if variant != "none":
        with np.errstate(under="ignore", over="ignore"):
            emaskT = np.ascontiguousarray(np.exp(mask).T.astype(np.float32))

    nc = _get_prog(variant)

    in_maps = []
    for c in range(NCORES):
        m = {
            "xT": xT,
            "wq": np.ascontiguousarray(
                wq_p[:, c * QH:(c + 1) * QH, :].reshape(DIM, DQ)),
            "wk": np.ascontiguousarray(wk_p[:, c, :]),
            "wv": np.ascontiguousarray(wv[:, c * HD:(c + 1) * HD]),
            "wo": np.ascontiguousarray(wo[c * DQ:(c + 1) * DQ, :]),
            "cosT": cosT,
            "sinT": sinT,
        }
        if variant != "none":
            m["emaskT"] = emaskT
        in_maps.append(m)
    return variant, nc, in_maps


def kernel(**inputs):
    global LAST_RESULTS
    variant, nc, in_maps = prepare(inputs)
    res = run_bass_kernel_spmd(nc, in_maps, core_ids=list(range(NCORES)))
    LAST_RESULTS = res
    out = res.results[0]["part"].astype(np.float64)
    for c in range(1, NCORES):
        out += res.results[c]["part"]
    return out.reshape(B, S, DIM).astype(np.float32)



# revision 42
# speedup vs baseline: 1.3841x; 1.3841x over previous
"""Trainium2 Bass kernel for GQA attention with RoPE (dense transformer block).

8-way tensor parallel over heads (KV head c, Q heads 4c..4c+3 per core);
host sums the 8 partial wo outputs (the all-reduce after wo).

Numerics/perf strategy (validated against the TimelineSim cost model and
real-HW fp8 DoubleRow behaviour):
  - QKV and wo matmuls: compensated fp8e4 DoubleRow — A@B ~= Ah@Bh + Al@Bh
    + Ah@Bl with hi/lo fp8 decompositions; 3 DR MMs cover a K=256 pair
    (residual ~0.3%).
  - Scores (QK^T): bf16 q/k after RoPE, single K=128 matmul per key tile.
  - exp(scores): fp8 for queries >= 512 (long causal rows average out the
    fp8 noise), bf16 for the first 512 queries; PV runs DoubleRow fp8
    (V hi+lo) resp. bf16; the softmax denominator is a ones-matmul over
    the same et data (self-normalizing).
  - Causal diagonal pairs restrict the query range to the needed half.
  - Partial outputs written fp16 with the 2^-14 descale folded on-chip.
"""
import math

import numpy as np
import ml_dtypes

import concourse.bass as bass
import concourse.tile as tile
from concourse import bacc, mybir
from concourse.bass_utils import run_bass_kernel_spmd
from concourse.masks import make_identity

B, S, DIM = 2, 2048, 4096
NH, NKV, HD = 32, 8, 128
BS = B * S
NCORES = 8
QH = NH // NCORES          # 4 q heads per core
DQ = QH * HD               # 512
TCH = 512                  # token chunk
NCH = BS // TCH            # 8 chunks
NKP = DIM // 256           # 16 k-pairs (DoubleRow planes)
NTP = BS // 256            # 16 V token-tile pairs
P = 128

F32 = mybir.dt.float32
BF16 = mybir.dt.bfloat16
F16 = mybir.dt.float16
FP8 = mybir.dt.float8e4
AF = mybir.ActivationFunctionType
DR = mybir.MatmulPerfMode.DoubleRow
E4NP = ml_dtypes.float8_e4m3
BFNP = ml_dtypes.bfloat16

XS = 16.0                  # x fp8 scale
WS = 1024.0                # weight fp8 scale
STG = 2.0 ** -10           # PSUM -> staged bf16 scale (16384*q -> 16*q)
ESC = 1.0 / (256.0 * math.sqrt(HD))   # scores psum -> true score
EB = -2.0                  # exp bias
OSC = 2.0 ** -14           # wo psum -> true out (16*1024*2^-14 = 1)

_prog_cache = {}
LAST_RESULTS = None


def _build(variant):
    nc = bacc.Bacc(None, target_bir_lowering=False)
    xh_d = nc.dram_tensor("xh", [DIM, BS], FP8, kind="ExternalInput")
    xl_d = nc.dram_tensor("xl", [DIM, BS], FP8, kind="ExternalInput")
    # 6 accs: qe0,qo0,qe1,qo1,k,v ; layout [acc, ki, jj, plane, mcol]
    wh_d = nc.dram_tensor("wh", [6, P, NKP, 2, P], FP8, kind="ExternalInput")
    wl_d = nc.dram_tensor("wl", [6, P, NKP, 2, P], FP8, kind="ExternalInput")
    woh_d = nc.dram_tensor("woh", [P, 2, 2, DIM], FP8, kind="ExternalInput")
    wol_d = nc.dram_tensor("wol", [P, 2, 2, DIM], FP8, kind="ExternalInput")
    cos_d = nc.dram_tensor("cos2", [P, S], BF16, kind="ExternalInput")
    sin_d = nc.dram_tensor("sin2", [P, S], BF16, kind="ExternalInput")
    m0f_d = nc.dram_tensor("m0f", [P, P], FP8, kind="ExternalInput")
    m0b_d = nc.dram_tensor("m0b", [P, P], BF16, kind="ExternalInput")
    emask_d = None
    if variant == "general":
        emask_d = nc.dram_tensor("emaskT", [S, S], BF16, kind="ExternalInput")
    part = nc.dram_tensor("part", [BS, DIM], F16, kind="ExternalOutput")

    causal = variant == "causal"
    xh3 = xh_d[:, :].rearrange("(jj pl ki) n -> ki jj pl n", jj=NKP, pl=2)
    xl3 = xl_d[:, :].rearrange("(jj pl ki) n -> ki jj pl n", jj=NKP, pl=2)
    uniq = [0]

    def _u():
        uniq[0] += 1
        return uniq[0]

    with tile.TileContext(nc) as tc:
        with (
            tc.tile_pool(name="const", bufs=1) as constp,
            tc.tile_pool(name="dram", bufs=1, space="DRAM") as dram,
            tc.tile_pool(name="kvp", bufs=1) as kvp,
            tc.tile_pool(name="opool", bufs=1) as opool,
        ):
            qTd = [dram.tile([QH, 2, 64, S], BF16, name=f"qTd{b}")
                   for b in range(B)]
            kTo = dram.tile([64, BS], BF16, name="kTo")

            bias_eb = constp.tile([P, 1], F32, name="bias_eb")
            nc.vector.memset(bias_eb[:], EB)
            ones8 = constp.tile([P, 2, P], FP8, name="ones8")
            nc.vector.memset(ones8[:], 1.0)
            onesb = constp.tile([P, P], BF16, name="onesb")
            nc.vector.memset(onesb[:], 1.0)
            identb = constp.tile([P, P], BF16, name="identb")
            make_identity(nc, identb)
            m0f = constp.tile([P, P], FP8, name="m0f")
            nc.sync.dma_start(m0f[:], m0f_d[:, :])
            m0b = constp.tile([P, P], BF16, name="m0b")
            nc.sync.dma_start(m0b[:], m0b_d[:, :])
            cos2 = constp.tile([P, S], BF16, name="cos2")
            sin2 = constp.tile([P, S], BF16, name="sin2")

            KT = kvp.tile([P, BS], BF16, name="KT")          # k^T [d, t]
            Vh = kvp.tile([P, NTP, 2, P], FP8, name="Vh")    # V tok-major hi
            Vl = kvp.tile([P, NTP, 2, P], FP8, name="Vl")    # V tok-major lo
            Vbf = kvp.tile([P, 2 * B, 2, P], BF16, name="Vbf")  # tt<4 per b
            O8h = [opool.tile([P, 2, 2, S], FP8, name=f"O8h{b}")
                   for b in range(B)]
            O8l = [opool.tile([P, 2, 2, S], FP8, name=f"O8l{b}")
                   for b in range(B)]

            # ================= Phase A: QKV + RoPE =================
            with (
                tc.tile_pool(name="wpool", bufs=1) as wpool,
                tc.tile_pool(name="xtp", bufs=1) as xtp,
                tc.tile_pool(name="stp", bufs=1) as stp,
                tc.tile_pool(name="psA", bufs=1, space="PSUM") as psA,
            ):
                wh_sb = [wpool.tile([P, NKP, 2, P], FP8, name=f"wh{m}")
                         for m in range(6)]
                wl_sb = [wpool.tile([P, NKP, 2, P], FP8, name=f"wl{m}")
                         for m in range(6)]
                for m in range(6):
                    nc.sync.dma_start(wh_sb[m][:, 0:4], wh_d[m, :, 0:4, :, :])
                for m in range(6):
                    nc.sync.dma_start(wl_sb[m][:, 0:4], wl_d[m, :, 0:4, :, :])
                nc.sync.dma_start(cos2[:], cos_d[:, :])
                nc.sync.dma_start(sin2[:], sin_d[:, :])
                wrest = [False]

                for tcn in range(NCH):
                    b, cb = tcn // (NCH // B), tcn % (NCH // B)
                    ch = slice(tcn * TCH, (tcn + 1) * TCH)
                    chc = slice(cb * TCH, (cb + 1) * TCH)  # within-batch pos
                    accs = [psA.tile([P, TCH], F32, tag="acc", bufs=7,
                                     name=f"acc{m}_{tcn}")
                            for m in range(6)]
                    xh_t, xl_t = {}, {}

                    def _load_grp(g, tcn=tcn, ch=ch):
                        for jj in range(g * 4, g * 4 + 4):
                            th = xtp.tile([P, 2, TCH], FP8, tag="xh", bufs=14,
                                          name=f"xh_{tcn}_{jj}")
                            nc.sync.dma_start(th[:], xh3[:, jj, :, ch])
                            tl = xtp.tile([P, 2, TCH], FP8, tag="xl", bufs=14,
                                          name=f"xl_{tcn}_{jj}")
                            nc.sync.dma_start(tl[:], xl3[:, jj, :, ch])
                            xh_t[jj] = th
                            xl_t[jj] = tl

                    _load_grp(0)
                    _load_grp(1)
                    if not wrest[0]:
                        wrest[0] = True
                        for m in range(6):
                            nc.sync.dma_start(wh_sb[m][:, 4:],
                                              wh_d[m, :, 4:, :, :])
                            nc.sync.dma_start(wl_sb[m][:, 4:],
                                              wl_d[m, :, 4:, :, :])
                    for grp in range(4):
                        if grp + 2 < 4:
                            _load_grp(grp + 2)
                        for m in range(6):
                            acc = accs[m]
                            for jj in range(grp * 4, grp * 4 + 4):
                                nc.tensor.matmul(
                                    acc[:], wh_sb[m][:, jj, :, :],
                                    xh_t[jj][:],
                                    start=(jj == 0), stop=False, perf_mode=DR)
                                nc.tensor.matmul(
                                    acc[:], wl_sb[m][:, jj, :, :],
                                    xh_t[jj][:],
                                    start=False, stop=False, perf_mode=DR)
                                nc.tensor.matmul(
                                    acc[:], wh_sb[m][:, jj, :, :],
                                    xl_t[jj][:],
                                    start=False, stop=(jj == NKP - 1),
                                    perf_mode=DR)

                    sts = {}
                    for m in range(6):
                        acc = accs[m]
                        if m == 4:
                            # k: stage even/odd halves to lane-aligned tiles
                            ke = stp.tile([64, TCH], BF16, tag="ke", bufs=2,
                                          name=f"ke_{tcn}")
                            nc.scalar.mul(ke[:], acc[0:64, :], STG)
                            ko = stp.tile([64, TCH], BF16, tag="ko", bufs=2,
                                          name=f"ko_{tcn}")
                            nc.scalar.mul(ko[:], acc[64:P, :], STG)
                            cs, sn = cos2[0:64, chc], sin2[0:64, chc]
                            kA = stp.tile([64, TCH], BF16, tag="kA", bufs=2,
                                          name=f"kA_{tcn}")
                            kB = stp.tile([64, TCH], BF16, tag="kB", bufs=2,
                                          name=f"kB_{tcn}")
                            nc.gpsimd.tensor_mul(kA[:], ke[:], cs)
                            nc.gpsimd.tensor_mul(kB[:], ko[:], sn)
                            nc.gpsimd.tensor_sub(KT[0:64, ch], kA[:], kB[:])
                            nc.gpsimd.tensor_mul(kA[:], ke[:], sn)
                            nc.gpsimd.tensor_mul(kB[:], ko[:], cs)
                            kob = stp.tile([64, TCH], BF16, tag="kob", bufs=2,
                                           name=f"kob_{tcn}")
                            nc.gpsimd.tensor_add(kob[:], kA[:], kB[:])
                            nc.sync.dma_start(kTo[:, ch], kob[:])
                            continue

                        st = stp.tile([P, TCH], BF16, tag="st", bufs=6,
                                      name=f"st{m}_{tcn}")
                        nc.scalar.mul(st[:], acc[:], STG)
                        sts[m] = st

                        if m in (1, 3):
                            # RoPE for q head-pair hp on DVE (bf16, 2x mode)
                            hp = (m - 1) // 2
                            se, so = sts[m - 1], sts[m]
                            cs, sn = cos2[:, chc], sin2[:, chc]
                            tA = stp.tile([P, TCH], BF16, tag="tA", bufs=2,
                                          name=f"tA{hp}_{tcn}")
                            tB = stp.tile([P, TCH], BF16, tag="tB", bufs=2,
                                          name=f"tB{hp}_{tcn}")
                            nc.vector.tensor_mul(tA[:], se[:], cs)
                            nc.vector.tensor_mul(tB[:], so[:], sn)
                            qe = stp.tile([P, TCH], BF16, tag="qe", bufs=2,
                                          name=f"qe{hp}_{tcn}")
                            nc.vector.tensor_sub(qe[:], tA[:], tB[:])
                            nc.vector.tensor_mul(tA[:], se[:], sn)
                            nc.vector.tensor_mul(tB[:], so[:], cs)
                            qo = stp.tile([P, TCH], BF16, tag="qo", bufs=2,
                                          name=f"qo{hp}_{tcn}")
                            nc.vector.tensor_add(qo[:], tA[:], tB[:])
                            for hh in range(2):
                                nc.sync.dma_start(
                                    qTd[b][2 * hp + hh, 0, :, chc],
                                    qe[64 * hh:64 * hh + 64, :])
                                nc.sync.dma_start(
                                    qTd[b][2 * hp + hh, 1, :, chc],
                                    qo[64 * hh:64 * hh + 64, :])
                        elif m == 5:
                            # V: transpose bf16 then hi/lo fp8 split
                            st_v = sts[5]
                            for j in range(TCH // P):
                                tp = psA.tile([P, P], BF16, tag="tp", bufs=1,
                                              name=f"tp_{tcn}_{j}")
                                nc.tensor.transpose(
                                    tp[:], st_v[:, j * P:(j + 1) * P],
                                    identb[:])
                                g = tcn * 4 + j
                                gp, pl = g // 2, g % 2
                                nc.scalar.copy(Vh[:, gp, pl, :], tp[:])
                                nc.vector.tensor_sub(
                                    Vl[:, gp, pl, :], tp[:], Vh[:, gp, pl, :])
                                if causal and cb == 0:
                                    nc.vector.tensor_copy(
                                        Vbf[:, b * 2 + j // 2, j % 2, :],
                                        tp[:])
                    if cb == (NCH // B) - 1:
                        nc.sync.dma_start(
                            KT[64:P, b * S:(b + 1) * S],
                            kTo[:, b * S:(b + 1) * S])

            # ================= Phases B and C =================
            with (
                tc.tile_pool(name="wop", bufs=1) as wop,
                tc.tile_pool(name="qtp", bufs=1) as qtp,
                tc.tile_pool(name="etp", bufs=1) as etp,
                tc.tile_pool(name="obp", bufs=1) as obp,
            ):
                psum_box = {}
                woh_sb = wop.tile([P, 2, 2, DIM], FP8, name="woh_sb")
                nc.sync.dma_start(woh_sb[:], woh_d[:, :, :, :])
                wol_sb = wop.tile([P, 2, 2, DIM], FP8, name="wol_sb")
                nc.sync.dma_start(wol_sb[:], wol_d[:, :, :, :])

                def emit_pv(b, o_ps, s_ps, et, pp, lo, bf_mode, last):
                    first = pp == 0
                    if bf_mode:
                        for u in range(2):
                            tt = 2 * pp + u
                            fin = last and u == 1
                            if causal:
                                vsl = Vbf[:, b * 2 + pp, u, :]
                            else:
                                g = b * (S // P) + tt
                                gp, pl = g // 2, g % 2
                                vt = etp.tile([P, P], BF16, tag="vtmp",
                                              bufs=2, name=f"vt_{_u()}")
                                nc.vector.tensor_add(
                                    vt[:], Vh[:, gp, pl, :], Vl[:, gp, pl, :])
                                vsl = vt[:]
                            nc.tensor.matmul(
                                o_ps[:, lo:TCH], vsl, et[:, u, lo:TCH],
                                start=(first and u == 0), stop=fin)
                            nc.tensor.matmul(
                                s_ps[:, lo:TCH], onesb[:], et[:, u, lo:TCH],
                                start=(first and u == 0), stop=fin)
                    else:
                        gp = b * (S // P) // 2 + pp
                        nc.tensor.matmul(
                            o_ps[:, lo:TCH], Vh[:, gp, :, :], et[:, :, lo:TCH],
                            start=first, stop=False, perf_mode=DR)
                        nc.tensor.matmul(
                            o_ps[:, lo:TCH], Vl[:, gp, :, :], et[:, :, lo:TCH],
                            start=False, stop=last, perf_mode=DR)
                        nc.tensor.matmul(
                            s_ps[:, lo:TCH], ones8[:], et[:, :, lo:TCH],
                            start=first, stop=last, perf_mode=DR)

                def load_qt(b, hg):
                    qts = []
                    for i in range(2):
                        h = 2 * hg + i
                        qt = qtp.tile([P, S], BF16, tag="qt", bufs=8,
                                      name=f"qt{b}_{h}")
                        nc.scalar.dma_start(qt[0:64, :], qTd[b][h, 0, :, :])
                        nc.scalar.dma_start(qt[64:P, :], qTd[b][h, 1, :, :])
                        qts.append(qt)
                    return qts

                def emit_B(b, hg, qts):
                    for i in range(2):
                        for sc in range(4):
                            bf_mode = (causal and sc == 0) or \
                                variant == "general"
                            npairs = (2 * sc + 2) if causal else 8
                            sch = slice(sc * TCH, (sc + 1) * TCH)
                            o_ps = psum_box["B"].tile([P, TCH], F32, tag="o", bufs=2,
                                            name=f"o_{b}_{hg}_{i}_{sc}")
                            s_ps = psum_box["B"].tile([P, TCH], F32, tag="s", bufs=2,
                                            name=f"s_{b}_{hg}_{i}_{sc}")
                            pend = []
                            for pp in range(npairs):
                                restr = causal and pp == npairs - 1
                                lo = TCH // 2 if restr else 0
                                scp = psum_box["B"].tile(
                                    [P, 2, TCH], F32, tag="scp", bufs=2,
                                    name=f"scp_{b}_{hg}_{i}_{sc}_{pp}")
                                for u in range(2):
                                    tt = 2 * pp + u
                                    nc.tensor.matmul(
                                        scp[:, u, lo:TCH],
                                        KT[:, b * S + tt * P:
                                           b * S + (tt + 1) * P],
                                        qts[i][:,
                                               sc * TCH + lo:(sc + 1) * TCH],
                                        start=True, stop=True)
                                if len(pend) >= 2:
                                    emit_pv(b, o_ps, s_ps, *pend.pop(0),
                                            last=False)
                                dt_ = BF16 if bf_mode else FP8
                                et = etp.tile(
                                    [P, 2, TCH], dt_,
                                    tag="etb" if bf_mode else "et8", bufs=6,
                                    name=f"et_{b}_{hg}_{i}_{sc}_{pp}")
                                nc.scalar.activation(
                                    et[:, :, lo:TCH], scp[:, :, lo:TCH],
                                    AF.Exp, bias=bias_eb[:], scale=ESC)
                                if variant == "general":
                                    mk = etp.tile(
                                        [P, 2, TCH], BF16, tag="mk", bufs=3,
                                        name=f"mk_{_u()}")
                                    for u in range(2):
                                        tt = 2 * pp + u
                                        nc.sync.dma_start(
                                            mk[:, u, :],
                                            emask_d[tt * P:(tt + 1) * P, sch])
                                    nc.vector.tensor_mul(et[:], et[:], mk[:])
                                elif causal and pp >= npairs - 2:
                                    m0 = m0b if bf_mode else m0f
                                    for u in range(2):
                                        j = 2 * (pp - (npairs - 2)) + u
                                        dlo = j * P
                                        nc.vector.tensor_mul(
                                            et[:, u, dlo:dlo + P],
                                            et[:, u, dlo:dlo + P], m0[:])
                                        if dlo > lo:
                                            nc.vector.memset(
                                                et[:, u, lo:dlo], 0.0)
                                pend.append((et, pp, lo, bf_mode))
                                yield
                            yield
                            while len(pend) > 1:
                                emit_pv(b, o_ps, s_ps, *pend.pop(0),
                                        last=False)
                                yield
                            emit_pv(b, o_ps, s_ps, *pend.pop(0), last=True)
                            rec = obp.tile([P, TCH], F32, tag="rec", bufs=2,
                                           name=f"rec_{b}_{hg}_{i}_{sc}")
                            nc.vector.reciprocal(rec[:], s_ps[:])
                            stO = obp.tile([P, TCH], BF16, tag="stO", bufs=2,
                                           name=f"stO_{b}_{hg}_{i}_{sc}")
                            nc.vector.tensor_mul(stO[:], o_ps[:], rec[:])
                            nc.vector.tensor_copy(O8h[b][:, hg, i, sch],
                                                  stO[:])
                            nc.gpsimd.tensor_sub(
                                O8l[b][:, hg, i, sch], stO[:],
                                O8h[b][:, hg, i, sch])
                            yield

                def emit_C(b, act_mod):
                    for tt in range(S // P):
                        tsl = slice(tt * P, (tt + 1) * P)
                        for hh in range(2):
                            ob = obp.tile([P, DIM // 2], F16, tag="ob",
                                          bufs=3, name=f"ob_{b}_{tt}_{hh}")
                            for n2 in range(DIM // TCH // 2):
                                nn = hh * (DIM // TCH // 2) + n2
                                pc = psum_box["C"].tile([P, TCH], F32, tag="pc",
                                              bufs=4,
                                              name=f"pc_{b}_{tt}_{nn}")
                                nsl = slice(nn * TCH, (nn + 1) * TCH)
                                osl = slice(n2 * TCH, (n2 + 1) * TCH)
                                for kk in range(2):
                                    nc.tensor.matmul(
                                        pc[:], O8h[b][:, kk, :, tsl],
                                        woh_sb[:, kk, :, nsl],
                                        start=(kk == 0), stop=False,
                                        perf_mode=DR)
                                    nc.tensor.matmul(
                                        pc[:], O8l[b][:, kk, :, tsl],
                                        woh_sb[:, kk, :, nsl],
                                        start=False, stop=False, perf_mode=DR)
                                    nc.tensor.matmul(
                                        pc[:], O8h[b][:, kk, :, tsl],
                                        wol_sb[:, kk, :, nsl],
                                        start=False, stop=(kk == 1),
                                        perf_mode=DR)
                                if nn % act_mod == 0:
                                    nc.scalar.mul(ob[:, osl], pc[:], OSC)
                                else:
                                    nc.vector.tensor_scalar_mul(
                                        ob[:, osl], pc[:], OSC)
                            nc.sync.dma_start(
                                part[b * S + tt * P:b * S + (tt + 1) * P,
                                     hh * (DIM // 2):(hh + 1) * (DIM // 2)],
                                ob[:])
                        yield

                # interleave B(0) and B(1) units to keep ACT (exp) saturated
                with tc.tile_pool(name="psB", bufs=1, space="PSUM") as psB:
                    psum_box["B"] = psB
                    qts_all = {(b, hg): load_qt(b, hg)
                               for hg in range(2) for b in range(B)}
                    for hg in range(2):
                        g0 = emit_B(0, hg, qts_all[(0, hg)])
                        g1 = emit_B(1, hg, qts_all[(1, hg)])
                        for _ in range(10):
                            next(g0, "end")
                        alive = True
                        while alive:
                            alive = False
                            if next(g0, "end") != "end":
                                alive = True
                            if next(g1, "end") != "end":
                                alive = True
                with tc.tile_pool(name="psC", bufs=1, space="PSUM") as psC:
                    psum_box["C"] = psC
                    for b in range(B):
                        for _ in emit_C(b, 2):
                            pass

    nc.compile()
    return nc


def _get_prog(variant):
    if variant not in _prog_cache:
        _prog_cache[variant] = _build(variant)
    return _prog_cache[variant]


def _comp8(a):
    """hi/lo fp8 e4m3 decomposition of an already-scaled fp32 array."""
    hi = a.astype(E4NP)
    lo = (a - hi.astype(np.float32)).astype(E4NP)
    return hi, lo


def prepare(inputs):
    x = np.asarray(inputs["x"], dtype=np.float32)
    wq = np.asarray(inputs["wq"], dtype=np.float32)
    wk = np.asarray(inputs["wk"], dtype=np.float32)
    wv = np.asarray(inputs["wv"], dtype=np.float32)
    wo = np.asarray(inputs["wo"], dtype=np.float32)
    fc = np.asarray(inputs["freqs_cos"], dtype=np.float32)
    fs = np.asarray(inputs["freqs_sin"], dtype=np.float32)
    mask = np.asarray(inputs["mask"], dtype=np.float32)

    xT = np.ascontiguousarray(x.reshape(BS, DIM).T) * XS
    xh, xl = _comp8(xT)

    even = np.arange(0, HD, 2)
    odd = np.arange(1, HD, 2)
    wq4 = wq.reshape(DIM, NH, HD)
    wk3 = wk.reshape(DIM, NKV, HD)
    cos2 = np.ascontiguousarray(
        np.concatenate([fc.T, fc.T], axis=0)).astype(BFNP)   # [128, S]
    sin2 = np.ascontiguousarray(
        np.concatenate([fs.T, fs.T], axis=0)).astype(BFNP)

    m0 = (np.arange(P)[None, :] >= np.arange(P)[:, None]).astype(np.float32)
    m0f = m0.astype(E4NP)
    m0b = m0.astype(BFNP)

    if not mask.any():
        variant = "none"
    else:
        il, jl = np.tril_indices(S)
        iu, ju = np.triu_indices(S, 1)
        if np.all(mask[il, jl] == 0.0) and np.all(mask[iu, ju] <= -1e8):
            variant = "causal"
        else:
            variant = "general"
    emaskT = None
    if variant == "general":
        with np.errstate(under="ignore", over="ignore"):
            emaskT = np.ascontiguousarray(np.exp(mask).T).astype(BFNP)

    nc = _get_prog(variant)

    in_maps = []
    for c in range(NCORES):
        accw = []
        for hp in range(2):
            h0, h1 = 4 * c + 2 * hp, 4 * c + 2 * hp + 1
            accw.append(np.concatenate(
                [wq4[:, h0, even], wq4[:, h1, even]], axis=1))
            accw.append(np.concatenate(
                [wq4[:, h0, odd], wq4[:, h1, odd]], axis=1))
        accw.append(np.concatenate(
            [wk3[:, c, even], wk3[:, c, odd]], axis=1))
        accw.append(wv[:, c * HD:(c + 1) * HD])
        whl = [np.ascontiguousarray(
            (a * WS).reshape(NKP, 2, P, P).transpose(2, 0, 1, 3))
            for a in accw]                       # [ki, jj, plane, mcol]
        pairs = [_comp8(a) for a in whl]
        wh = np.stack([p[0] for p in pairs])
        wl = np.stack([p[1] for p in pairs])

        wo_c = wo[c * DQ:(c + 1) * DQ, :] * WS   # [512, DIM]
        wo_r = np.ascontiguousarray(
            wo_c.reshape(2, 2, P, DIM).transpose(2, 0, 1, 3))
        woh, wol = _comp8(wo_r)

        m = {
            "xh": xh, "xl": xl,
            "wh": wh, "wl": wl,
            "woh": woh, "wol": wol,
            "cos2": cos2, "sin2": sin2,
            "m0f": m0f, "m0b": m0b,
        }
        if variant == "general":
            m["emaskT"] = emaskT
        in_maps.append(m)
    return variant, nc, in_maps


def kernel(**inputs):
    global LAST_RESULTS
    variant, nc, in_maps = prepare(inputs)
    res = run_bass_kernel_spmd(nc, in_maps, core_ids=list(range(NCORES)))
    LAST_RESULTS = res
    out = res.results[0]["part"].astype(np.float64)
    for c in range(1, NCORES):
        out += res.results[c]["part"]
    return out.reshape(B, S, DIM).astype(np.float32)
